# revision 12
# baseline (speedup 1.0000x reference)
"""KAN layer kernel for 8x Trainium2 NeuronCores — fp8-DoubleRow hybrid.

y[n,k] = sum_{j,i} exp(-16*(x[n,i]*bw[j,i]+bb[j,i])^2) * W[k,j,i]
         + bias[k] + cos(x) @ scale_base.T

Sharding: data-parallel over N (8192 rows -> 1024 rows/core), params
replicated.

Precision plan (per input feature i, the 16 j-basis pairs are ranked by
fp8-quantization error mass, estimated on a row subsample):
  - top T=4 slots  -> bf16 chunks (exact-ish), standard matmuls
  - remaining 12   -> fp8 e4m3, packed 2 slots/super-chunk, DoubleRow
    matmuls (2x contraction per PE pass)
  - W for fp8 slots is quantized with batch-adaptive coordinated rounding
    (AdaRound-style greedy minimizing E_n[(sum_j bhat_j dW_j)^2] per (i,k))
  - base path cos(x) @ scale_base.T stays bf16
All matmuls accumulate f32 into shared PSUM banks; W (and scale_base) are
pre-scaled by S=1024 so fp8 W values sit in e4m3's normal range; the 1/S
dequant is folded into the PSUM->SBUF copy.

Engine balance: PE is the bottleneck (~306us). Exp passes must run on ACT;
Square passes are routed across ACT / Pool(+DVE) / Pool(+Pool) so no
helper engine exceeds the PE time. cos runs on DVE in [128,2048] passes.
"""

import sys

for _p in ("/opt/trn_rl_repo",):
    if _p not in sys.path:
        sys.path.insert(0, _p)

import math

import ml_dtypes
import numpy as np

import concourse.bass as bass
import concourse.mybir as mybir
import concourse.tile as tile
from concourse import bacc
from concourse.bass_utils import run_bass_kernel_spmd

F32 = mybir.dt.float32
BF16 = mybir.dt.bfloat16
F8 = mybir.dt.float8e4
AF = mybir.ActivationFunctionType
ALU = mybir.AluOpType
PM = mybir.MatmulPerfMode
F8NP = ml_dtypes.float8_e4m3

N_CORES = 8
N, IN, OUT, NB = 8192, 1024, 1024, 16
NSH = N // N_CORES            # rows per core = 1024
ICHUNK = IN // 128            # 8 i-chunks
RB = 2                        # row blocks per core
RBW = NSH // RB               # 512 rows per block
MT = RBW // 128               # 4 m-tiles per block
S = 1024.0                    # fp8 W scale

T_BF = 4                      # bf16 slots per i
NU = (NB - T_BF) // 2         # fp8 super-chunk units per ic = 6
NCHK = NB * ICHUNK            # 128 chunk-slots total

TWO_PI = 2.0 * math.pi
MAGIC = 12582912.0            # 1.5 * 2**23
CC = [
    0.9999992107823226,
    -0.49999421338471783,
    0.04165977780655192,
    -0.0013858789919604375,
    2.420294136739255e-05,
    -2.1972963819539338e-07,
]

# square-pass routing for fp8 chunks: A = ACT Square; PD = Pool affine +
# DVE mult; PP = Pool affine + Pool mult
ROUTE_PATTERN = ("PD", "PP", "PD", "A")

_cache = {}


def _route(sc_chunk_idx):
    return ROUTE_PATTERN[sc_chunk_idx % len(ROUTE_PATTERN)]


def _build():
    nc = bacc.Bacc("TRN2", target_bir_lowering=False)

    x_t = nc.dram_tensor("x_t", [IN, NSH], F32, kind="ExternalInput")
    w_bf = nc.dram_tensor("w_bf", [ICHUNK * T_BF * 128, OUT], BF16,
                          kind="ExternalInput")
    w_f8 = nc.dram_tensor("w_f8", [ICHUNK * NU * 256, OUT], F8,
                          kind="ExternalInput")
    sb_t = nc.dram_tensor("sb_t", [IN, OUT], BF16, kind="ExternalInput")
    bw4 = nc.dram_tensor("bw4", [128, NCHK], F32, kind="ExternalInput")
    bb4 = nc.dram_tensor("bb4", [128, NCHK], F32, kind="ExternalInput")
    bias_f = nc.dram_tensor("bias_f", [1, OUT], F32, kind="ExternalInput")
    y = nc.dram_tensor("y", [NSH, OUT], F32, kind="ExternalOutput")

    with tile.TileContext(nc) as tc:
        with (
            tc.tile_pool(name="singles", bufs=1) as singles,
            tc.tile_pool(name="wpool", bufs=5) as wpool,
            tc.tile_pool(name="w8pool", bufs=5) as w8pool,
            tc.tile_pool(name="sqpool", bufs=3) as sqpool,
            tc.tile_pool(name="sq8pool", bufs=3) as sq8pool,
            tc.tile_pool(name="tpool", bufs=3) as tpool,
            tc.tile_pool(name="bpool", bufs=3) as bpool,
            tc.tile_pool(name="b8pool", bufs=3) as b8pool,
            tc.tile_pool(name="ypool", bufs=3) as ypool,
            tc.tile_pool(name="tmp", bufs=1) as tmp,
            tc.tile_pool(name="psum", bufs=1, space="PSUM") as psum,
        ):
            # tiny per-chunk scale/bias tables first on the sync ring
            bw4_sb = singles.tile([128, NCHK], F32)
            nc.sync.dma_start(out=bw4_sb, in_=bw4[:])
            bb4_sb = singles.tile([128, NCHK], F32)
            nc.sync.dma_start(out=bb4_sb, in_=bb4[:])

            # x^T resident; ic0 split sync+scalar for earliest start, rest
            # round-robin on scalar/vector rings
            xt_sb = singles.tile([128, ICHUNK, NSH], F32)
            xt_dram = x_t[:].rearrange("(c p) n -> p c n", p=128)
            nc.sync.dma_start(out=xt_sb[:, 0, :RBW], in_=xt_dram[:, 0, :RBW])
            nc.scalar.dma_start(out=xt_sb[:, 0, RBW:], in_=xt_dram[:, 0, RBW:])
            nc.sync.dma_start(out=xt_sb[:, 2, :], in_=xt_dram[:, 2, :])
            for ic in (1, 3, 4, 5, 6, 7):
                nc.gpsimd.dma_start(out=xt_sb[:, ic, :], in_=xt_dram[:, ic, :])

            sbt_sb = singles.tile([128, ICHUNK, OUT], BF16)
            sbt_dram = sb_t[:].rearrange("(c p) n -> p c n", p=128)
            for c in range(0, ICHUNK, 2):
                nc.gpsimd.dma_start(
                    out=sbt_sb[:, c : c + 2, :], in_=sbt_dram[:, c : c + 2, :]
                )
            bias_bc = singles.tile([128, OUT], F32)
            nc.gpsimd.dma_start(out=bias_bc, in_=bias_f[:].to_broadcast([128, OUT]))
            cosx_sb = singles.tile([128, ICHUNK, NSH], BF16)

            sc_sq_counter = [0]

            def emit_square(dst, xs, cid, rb, name):
                """dst[128, RBW] f32 = (bw4*x + bb4)^2 via routed engines."""
                r = _route(sc_sq_counter[0])
                sc_sq_counter[0] += 1
                if r == "A":
                    nc.scalar.activation(
                        dst, xs, AF.Square,
                        bias=bb4_sb[:, cid : cid + 1],
                        scale=bw4_sb[:, cid : cid + 1],
                    )
                else:
                    t = tpool.tile([128, RBW], F32, tag="t", name=f"t{name}")
                    nc.gpsimd.tensor_scalar(
                        t, xs, bw4_sb[:, cid : cid + 1],
                        bb4_sb[:, cid : cid + 1], ALU.mult, ALU.add,
                    )
                    eng = nc.vector if r == "PD" else nc.gpsimd
                    eng.tensor_tensor(dst, t, t, ALU.mult)

            def spline_units(rb):
                ns = rb * RBW
                ps = [
                    [
                        psum.tile(
                            [128, 512], F32,
                            tag=f"ps_{mt}_{ob}", name=f"ps_{rb}_{mt}_{ob}",
                        )
                        for ob in range(2)
                    ]
                    for mt in range(MT)
                ]
                first = [True]

                def mms_bf(bas, wt):
                    for mt in range(MT):
                        lhsT = bas[:, mt * 128 : (mt + 1) * 128]
                        for ob in range(2):
                            nc.tensor.matmul(
                                ps[mt][ob], lhsT,
                                wt[:, ob * 512 : (ob + 1) * 512],
                                start=first[0], stop=False,
                            )
                    first[0] = False

                def mms_dr(bas8, wt8):
                    for mt in range(MT):
                        lhsT = bas8[:, :, mt * 128 : (mt + 1) * 128]
                        for ob in range(2):
                            nc.tensor.matmul(
                                ps[mt][ob], lhsT,
                                wt8[:, :, ob * 512 : (ob + 1) * 512],
                                start=first[0], stop=False,
                                perf_mode=PM.DoubleRow,
                            )
                    first[0] = False

                def bf_unit(ic, t_):
                    uid = ic * T_BF + t_
                    cid = ic * NB + t_
                    wt = wpool.tile([128, OUT], BF16, tag="wt",
                                    name=f"wt{rb}_{uid}")
                    nc.sync.dma_start(
                        out=wt, in_=w_bf[uid * 128 : (uid + 1) * 128, :]
                    )
                    sq = sqpool.tile([128, RBW], F32, tag="sq",
                                     name=f"sq{rb}_{uid}")
                    nc.scalar.activation(
                        sq, xt_sb[:, ic, ns : ns + RBW], AF.Square,
                        bias=bb4_sb[:, cid : cid + 1],
                        scale=bw4_sb[:, cid : cid + 1],
                    )
                    bas = bpool.tile([128, RBW], BF16, tag="bas",
                                     name=f"bas{rb}_{uid}")
                    nc.scalar.activation(bas, sq, AF.Exp, scale=-1.0)
                    mms_bf(bas, wt)

                def sc_unit(ic, u):
                    uid = ic * NU + u
                    wt8 = w8pool.tile([128, 2, OUT], F8, tag="wt8",
                                      name=f"wt8_{rb}_{uid}")
                    nc.sync.dma_start(
                        out=wt8,
                        in_=w_f8[uid * 256 : (uid + 1) * 256, :].rearrange(
                            "(two p) o -> p two o", two=2
                        ),
                    )
                    sq8 = sq8pool.tile([128, 2, RBW], F32, tag="sq8",
                                       name=f"sq8_{rb}_{uid}")
                    xs = xt_sb[:, ic, ns : ns + RBW]
                    for pl in range(2):
                        cid = ic * NB + T_BF + 2 * u + pl
                        emit_square(sq8[:, pl, :], xs, cid, rb,
                                    f"{rb}_{uid}_{pl}")
                    bas8 = b8pool.tile([128, 2, RBW], F8, tag="bas8",
                                       name=f"bas8_{rb}_{uid}")
                    nc.scalar.activation(bas8, sq8, AF.Exp, scale=-1.0)
                    mms_dr(bas8, wt8)

                # interleave bf (ACT-heavy) and sc (pool/DVE-heavy) units
                for ic in range(ICHUNK):
                    seq = [("b", 0), ("s", 0), ("s", 1), ("b", 1), ("s", 2),
                           ("s", 3), ("b", 2), ("s", 4), ("b", 3), ("s", 5)]
                    for kind, idx in seq:
                        if kind == "b":
                            bf_unit(ic, idx)
                        else:
                            sc_unit(ic, idx)
                return ps

            def base_and_out(rb, ps):
                ns = rb * RBW
                for mt in range(MT):
                    for bc in range(ICHUNK):
                        last = bc == ICHUNK - 1
                        lhsT = cosx_sb[:, bc, ns + mt * 128 : ns + (mt + 1) * 128]
                        for ob in range(2):
                            nc.tensor.matmul(
                                ps[mt][ob], lhsT,
                                sbt_sb[:, bc, ob * 512 : (ob + 1) * 512],
                                start=False, stop=last,
                            )
                    y_sb = ypool.tile([128, OUT], F32, tag="y",
                                      name=f"y{rb}_{mt}")
                    for ob in range(2):
                        nc.vector.scalar_tensor_tensor(
                            y_sb[:, ob * 512 : (ob + 1) * 512],
                            ps[mt][ob], 1.0 / S,
                            bias_bc[:, ob * 512 : (ob + 1) * 512],
                            ALU.mult, ALU.add,
                        )
                    r0 = ns + mt * 128
                    (nc.sync, nc.gpsimd)[mt % 2].dma_start(
                        out=y[r0 : r0 + 128, :], in_=y_sb
                    )

            # ---- rb0 spline stream ----
            ps0 = spline_units(0)

            # ---- cos path on DVE, [128, 2048] mega-tiles ----
            for g in range(ICHUNK // 2):
                xs = xt_sb[:, 2 * g : 2 * g + 2, :]
                t1 = tmp.tile([128, 2, NSH], F32, tag="t1", name=f"t1_{g}")
                nc.vector.tensor_scalar_mul(t1, xs, 1.0 / TWO_PI)
                t2 = tmp.tile([128, 2, NSH], F32, tag="t2", name=f"t2_{g}")
                nc.vector.tensor_scalar_add(t2, t1, MAGIC)
                nc.vector.tensor_scalar_sub(t1, t2, MAGIC)
                nc.vector.tensor_scalar_mul(t2, t1, -TWO_PI)
                r = tmp.tile([128, 2, NSH], F32, tag="r", name=f"r_{g}")
                nc.vector.tensor_add(r, xs, t2)
                u = tmp.tile([128, 2, NSH], F32, tag="u", name=f"u_{g}")
                nc.vector.tensor_mul(u, r, r)
                nc.vector.tensor_scalar_mul(t1, u, CC[5])
                nc.vector.scalar_tensor_tensor(t2, t1, CC[4], u, ALU.add, ALU.mult)
                nc.vector.scalar_tensor_tensor(t1, t2, CC[3], u, ALU.add, ALU.mult)
                nc.vector.scalar_tensor_tensor(t2, t1, CC[2], u, ALU.add, ALU.mult)
                nc.vector.scalar_tensor_tensor(t1, t2, CC[1], u, ALU.add, ALU.mult)
                nc.vector.tensor_scalar_add(
                    cosx_sb[:, 2 * g : 2 * g + 2, :], t1, CC[0]
                )

            # ---- rb0 base + out, rb1 ----
            base_and_out(0, ps0)
            ps1 = spline_units(1)
            base_and_out(1, ps1)

    nc.compile()
    return nc


def _fp8_neighbors(v):
    """Down/up e4m3 grid neighbors of v (f32)."""
    r = v.astype(F8NP).astype(np.float32)
    eps = (np.maximum(np.abs(r), 2.0 ** -9) * 2.0 ** -4).astype(np.float32)
    lo = np.where(r > v, (v - eps).astype(F8NP).astype(np.float32), r)
    hi = np.where(r <= v, (v + eps).astype(F8NP).astype(np.float32), r)
    return np.minimum(lo, r), np.maximum(hi, r)


def _global_prep(inputs):
    """Rank slots, AdaRound fp8 W, pack device arrays (shared by all cores)."""
    x = np.asarray(inputs["x"], np.float32)
    bwf = np.asarray(inputs["basis_w"], np.float32)
    bbf = np.asarray(inputs["basis_b"], np.float32)
    W = np.asarray(inputs["W"], np.float32)
    bias = np.asarray(inputs["bias"], np.float32)
    sbm = np.asarray(inputs["scale_base"], np.float32)

    ns = 1024
    xs = x[:: max(1, x.shape[0] // ns)][:ns]
    t = xs[:, None, :] * bwf[None] + bbf[None]
    b = np.exp(-16.0 * t * t).astype(np.float32)
    b8 = b.astype(F8NP).astype(np.float32)

    Wt = np.ascontiguousarray(W.transpose(1, 2, 0))      # [NB, IN, OUT]
    db2 = ((b8 - b) ** 2).mean(0)                        # [NB, IN]
    sumW2 = (Wt.astype(np.float64) ** 2).sum(2).astype(np.float32)
    mass = db2 * sumW2
    order = np.argsort(-mass, axis=0)                    # [NB, IN]

    Wp = np.take_along_axis(Wt, order[:, :, None], axis=0)
    bwp = np.take_along_axis(bwf, order, axis=0)
    bbp = np.take_along_axis(bbf, order, axis=0)
    b8p = np.take_along_axis(b8, order[None], axis=1)    # [ns, NB, IN]

    # AdaRound for fp8 slots
    WS = (Wp.astype(np.float64) * S).astype(np.float32)
    mhat = b8p.copy()
    mhat[:, :T_BF, :] = 0.0
    m = np.ascontiguousarray(mhat.transpose(2, 1, 0))    # [IN, NB, ns]
    C = np.einsum("ijn,ikn->ijk", m, m, optimize=True) / ns
    Ci = np.ascontiguousarray(C.transpose(1, 2, 0))      # [NB, NB, IN]
    lo, hi = _fp8_neighbors(WS)
    rne = WS.astype(F8NP).astype(np.float32)
    dlo = lo - WS
    dhi = hi - WS
    d = np.where(np.abs(rne - hi) < np.abs(rne - lo), dhi, dlo)
    d[:T_BF] = 0.0
    for _ in range(3):
        for j in range(T_BF, NB):
            s = np.einsum("li,lik->ik", Ci[j], d, optimize=True) \
                - Ci[j, j][:, None] * d[j]
            Cjj = Ci[j, j][:, None]
            cost_lo = Cjj * dlo[j] ** 2 + 2.0 * dlo[j] * s
            cost_hi = Cjj * dhi[j] ** 2 + 2.0 * dhi[j] * s
            d[j] = np.where(cost_hi < cost_lo, dhi[j], dlo[j])
    W8 = WS + d                                          # fp8 grid values

    # pack unit streams (ic-major)
    w_bf = np.empty((ICHUNK * T_BF * 128, OUT), dtype=ml_dtypes.bfloat16)
    for ic in range(ICHUNK):
        blk = slice(ic * 128, (ic + 1) * 128)
        for t_ in range(T_BF):
            uid = ic * T_BF + t_
            w_bf[uid * 128 : (uid + 1) * 128] = (
                Wp[t_, blk, :] * np.float32(S)
            ).astype(ml_dtypes.bfloat16)
    w_f8 = np.empty((ICHUNK * NU * 256, OUT), dtype=F8NP)
    for ic in range(ICHUNK):
        blk = slice(ic * 128, (ic + 1) * 128)
        for u in range(NU):
            uid = ic * NU + u
            for pl in range(2):
                rows = slice(uid * 256 + pl * 128, uid * 256 + (pl + 1) * 128)
                w_f8[rows] = W8[T_BF + 2 * u + pl, blk, :].astype(F8NP)

    # per-chunk scale/bias tables: cid = ic*NB + slot
    bw4 = np.empty((128, NCHK), np.float32)
    bb4 = np.empty((128, NCHK), np.float32)
    for ic in range(ICHUNK):
        blk = slice(ic * 128, (ic + 1) * 128)
        for slot in range(NB):
            cid = ic * NB + slot
            bw4[:, cid] = 4.0 * bwp[slot, blk]
            bb4[:, cid] = 4.0 * bbp[slot, blk]

    sb_t = np.ascontiguousarray(sbm.T * np.float32(S)).astype(ml_dtypes.bfloat16)
    bias_f = np.ascontiguousarray(bias.reshape(1, OUT))
    return {
        "w_bf": w_bf,
        "w_f8": w_f8,
        "sb_t": sb_t,
        "bw4": np.ascontiguousarray(bw4),
        "bb4": np.ascontiguousarray(bb4),
        "bias_f": bias_f,
    }


def _prep(inputs):
    key = None
    xarr = np.asarray(inputs["x"])
    key = (xarr.shape, xarr.dtype.str, xarr[:2, :4].tobytes())
    if _cache.get("prep_key") != key:
        _cache["prep"] = _global_prep(inputs)
        _cache["prep_key"] = key
    shared = _cache["prep"]

    x = np.asarray(inputs["x"], dtype=np.float32)
    in_maps = []
    for c in range(N_CORES):
        shard = x[c * NSH : (c + 1) * NSH, :]
        x_t = np.ascontiguousarray(shard.T)
        im = {"x_t": x_t}
        im.update(shared)
        in_maps.append(im)
    return in_maps


def run(inputs, trace=False, **kw):
    if "nc" not in _cache:
        _cache["nc"] = _build()
    nc = _cache["nc"]
    in_maps = _prep(inputs)
    res = run_bass_kernel_spmd(
        nc, in_maps, core_ids=list(range(N_CORES)), trace=trace, **kw
    )
    out = np.concatenate([res.results[c]["y"] for c in range(N_CORES)], axis=0)
    return out, res


def kernel(**inputs) -> np.ndarray:
    out, _ = run(inputs, trace=False)
    return out


# revision 16
# speedup vs baseline: 1.0038x; 1.0038x over previous
"""KAN layer kernel for 8x Trainium2 NeuronCores — fp8-DoubleRow hybrid.

y[n,k] = sum_{j,i} exp(-16*(x[n,i]*bw[j,i]+bb[j,i])^2) * W[k,j,i]
         + bias[k] + cos(x) @ scale_base.T

Sharding: data-parallel over N (8192 rows -> 1024 rows/core), params
replicated.

Precision plan (per input feature i, the 16 j-basis pairs are ranked by
fp8-quantization error mass, estimated on a row subsample):
  - top T=4 slots  -> bf16 chunks (exact-ish), standard matmuls
  - remaining 12   -> fp8 e4m3, packed 2 slots/super-chunk, DoubleRow
    matmuls (2x contraction per PE pass)
  - W for fp8 slots is quantized with batch-adaptive coordinated rounding
    (AdaRound-style greedy minimizing E_n[(sum_j bhat_j dW_j)^2] per (i,k))
  - base path cos(x) @ scale_base.T stays bf16
All matmuls accumulate f32 into shared PSUM banks; W (and scale_base) are
pre-scaled by S=1024 so fp8 W values sit in e4m3's normal range; the 1/S
dequant is folded into the PSUM->SBUF copy.

Engine balance: PE is the bottleneck (~306us). Exp passes must run on ACT;
Square passes are routed across ACT / Pool(+DVE) / Pool(+Pool) so no
helper engine exceeds the PE time. cos runs on DVE in [128,2048] passes.
"""

import sys

for _p in ("/opt/trn_rl_repo",):
    if _p not in sys.path:
        sys.path.insert(0, _p)

import math

import ml_dtypes
import numpy as np

import concourse.bass as bass
import concourse.mybir as mybir
import concourse.tile as tile
from concourse import bacc
from concourse.bass_utils import run_bass_kernel_spmd

F32 = mybir.dt.float32
BF16 = mybir.dt.bfloat16
F8 = mybir.dt.float8e4
AF = mybir.ActivationFunctionType
ALU = mybir.AluOpType
PM = mybir.MatmulPerfMode
F8NP = ml_dtypes.float8_e4m3

N_CORES = 8
N, IN, OUT, NB = 8192, 1024, 1024, 16
NSH = N // N_CORES            # rows per core = 1024
ICHUNK = IN // 128            # 8 i-chunks
RB = 2                        # row blocks per core
RBW = NSH // RB               # 512 rows per block
MT = RBW // 128               # 4 m-tiles per block
S = 1024.0                    # fp8 W scale

T_BF = 4                      # bf16 slots per i
NU = (NB - T_BF) // 2         # fp8 super-chunk units per ic = 6
NCHK = NB * ICHUNK            # 128 chunk-slots total

TWO_PI = 2.0 * math.pi
MAGIC = 12582912.0            # 1.5 * 2**23
CC = [
    0.9999992107823226,
    -0.49999421338471783,
    0.04165977780655192,
    -0.0013858789919604375,
    2.420294136739255e-05,
    -2.1972963819539338e-07,
]

# square-pass routing for fp8 chunks: A = ACT Square; PD = Pool affine +
# DVE mult; PP = Pool affine + Pool mult
ROUTE_PATTERN = ("PD", "PP", "PD", "A")

_cache = {}


def _route(sc_chunk_idx):
    return ROUTE_PATTERN[sc_chunk_idx % len(ROUTE_PATTERN)]


def _build():
    nc = bacc.Bacc("TRN2", target_bir_lowering=False)

    x_t = nc.dram_tensor("x_t", [IN, NSH], F32, kind="ExternalInput")
    w_bf = nc.dram_tensor("w_bf", [ICHUNK * T_BF * 128, OUT], BF16,
                          kind="ExternalInput")
    w_f8 = nc.dram_tensor("w_f8", [ICHUNK * NU * 256, OUT], F8,
                          kind="ExternalInput")
    sb_t = nc.dram_tensor("sb_t", [IN, OUT], BF16, kind="ExternalInput")
    bw4 = nc.dram_tensor("bw4", [128, NCHK], F32, kind="ExternalInput")
    bb4 = nc.dram_tensor("bb4", [128, NCHK], F32, kind="ExternalInput")
    bias_f = nc.dram_tensor("bias_f", [1, OUT], F32, kind="ExternalInput")
    y = nc.dram_tensor("y", [NSH, OUT], F32, kind="ExternalOutput")

    with tile.TileContext(nc) as tc:
        with (
            tc.tile_pool(name="singles", bufs=1) as singles,
            tc.tile_pool(name="wpool", bufs=5) as wpool,
            tc.tile_pool(name="w8pool", bufs=5) as w8pool,
            tc.tile_pool(name="sqpool", bufs=3) as sqpool,
            tc.tile_pool(name="sq8pool", bufs=3) as sq8pool,
            tc.tile_pool(name="tpool", bufs=3) as tpool,
            tc.tile_pool(name="bpool", bufs=3) as bpool,
            tc.tile_pool(name="b8pool", bufs=3) as b8pool,
            tc.tile_pool(name="ypool", bufs=3) as ypool,
            tc.tile_pool(name="tmp", bufs=1) as tmp,
            tc.tile_pool(name="psum", bufs=1, space="PSUM") as psum,
        ):
            # tiny per-chunk scale/bias tables first on the sync ring
            bw4_sb = singles.tile([128, NCHK], F32)
            nc.sync.dma_start(out=bw4_sb, in_=bw4[:])
            bb4_sb = singles.tile([128, NCHK], F32)
            nc.sync.dma_start(out=bb4_sb, in_=bb4[:])

            # x^T resident as one tile per PAIR of i-chunks so consumers only
            # wait on their own pair's DMAs (a single 4MB tile serializes the
            # first Square behind the whole x load); pairs keep the cos path's
            # [128, 2048] DVE passes possible
            xt_dram = x_t[:].rearrange("(c p) n -> p c n", p=128)
            xt_sb = []
            for g in range(ICHUNK // 2):
                xt_sb.append(singles.tile([128, 2, NSH], F32, name=f"xt{g}"))

            def xs_of(ic):
                return xt_sb[ic // 2][:, ic % 2, :]

            nc.sync.dma_start(out=xt_sb[0][:, 0, :RBW], in_=xt_dram[:, 0, :RBW])
            nc.scalar.dma_start(out=xt_sb[0][:, 0, RBW:], in_=xt_dram[:, 0, RBW:])
            rings = (nc.gpsimd, nc.scalar)
            for ic in range(1, ICHUNK):
                rings[ic % 2].dma_start(
                    out=xs_of(ic), in_=xt_dram[:, ic, :]
                )

            sbt_dram = sb_t[:].rearrange("(c p) n -> p c n", p=128)
            sbt_sb = []
            for c in range(ICHUNK):
                sbt_sb.append(singles.tile([128, OUT], BF16, name=f"sbt{c}"))
                nc.gpsimd.dma_start(out=sbt_sb[c], in_=sbt_dram[:, c, :])
            bias_bc = singles.tile([128, OUT], F32)
            nc.gpsimd.dma_start(out=bias_bc, in_=bias_f[:].to_broadcast([128, OUT]))
            cosx_sb = []
            for g in range(ICHUNK // 2):
                cosx_sb.append(singles.tile([128, 2, NSH], BF16, name=f"cosx{g}"))

            sc_sq_counter = [0]

            def emit_square(dst, xs, cid, rb, name):
                """dst[128, RBW] f32 = (bw4*x + bb4)^2 via routed engines."""
                r = _route(sc_sq_counter[0])
                sc_sq_counter[0] += 1
                if r == "A":
                    nc.scalar.activation(
                        dst, xs, AF.Square,
                        bias=bb4_sb[:, cid : cid + 1],
                        scale=bw4_sb[:, cid : cid + 1],
                    )
                else:
                    t = tpool.tile([128, RBW], F32, tag="t", name=f"t{name}")
                    nc.gpsimd.tensor_scalar(
                        t, xs, bw4_sb[:, cid : cid + 1],
                        bb4_sb[:, cid : cid + 1], ALU.mult, ALU.add,
                    )
                    eng = nc.vector if r == "PD" else nc.gpsimd
                    eng.tensor_tensor(dst, t, t, ALU.mult)

            def spline_units(rb):
                ns = rb * RBW
                ps = [
                    [
                        psum.tile(
                            [128, 512], F32,
                            tag=f"ps_{mt}_{ob}", name=f"ps_{rb}_{mt}_{ob}",
                        )
                        for ob in range(2)
                    ]
                    for mt in range(MT)
                ]
                first = [True]

                def mms_bf(bas, wt):
                    for mt in range(MT):
                        lhsT = bas[:, mt * 128 : (mt + 1) * 128]
                        for ob in range(2):
                            nc.tensor.matmul(
                                ps[mt][ob], lhsT,
                                wt[:, ob * 512 : (ob + 1) * 512],
                                start=first[0], stop=False,
                            )
                    first[0] = False

                def mms_dr(bas8, wt8):
                    for mt in range(MT):
                        lhsT = bas8[:, :, mt * 128 : (mt + 1) * 128]
                        for ob in range(2):
                            nc.tensor.matmul(
                                ps[mt][ob], lhsT,
                                wt8[:, :, ob * 512 : (ob + 1) * 512],
                                start=first[0], stop=False,
                                perf_mode=PM.DoubleRow,
                            )
                    first[0] = False

                def bf_unit(ic, t_):
                    uid = ic * T_BF + t_
                    cid = ic * NB + t_
                    wt = wpool.tile([128, OUT], BF16, tag="wt",
                                    name=f"wt{rb}_{uid}")
                    nc.sync.dma_start(
                        out=wt, in_=w_bf[uid * 128 : (uid + 1) * 128, :]
                    )
                    sq = sqpool.tile([128, RBW], F32, tag="sq",
                                     name=f"sq{rb}_{uid}")
                    nc.scalar.activation(
                        sq, xs_of(ic)[:, ns : ns + RBW], AF.Square,
                        bias=bb4_sb[:, cid : cid + 1],
                        scale=bw4_sb[:, cid : cid + 1],
                    )
                    bas = bpool.tile([128, RBW], BF16, tag="bas",
                                     name=f"bas{rb}_{uid}")
                    nc.scalar.activation(bas, sq, AF.Exp, scale=-1.0)
                    mms_bf(bas, wt)

                def sc_unit(ic, u):
                    uid = ic * NU + u
                    wt8 = w8pool.tile([128, 2, OUT], F8, tag="wt8",
                                      name=f"wt8_{rb}_{uid}")
                    nc.sync.dma_start(
                        out=wt8,
                        in_=w_f8[uid * 256 : (uid + 1) * 256, :].rearrange(
                            "(two p) o -> p two o", two=2
                        ),
                    )
                    sq8 = sq8pool.tile([128, 2, RBW], F32, tag="sq8",
                                       name=f"sq8_{rb}_{uid}")
                    xs = xs_of(ic)[:, ns : ns + RBW]
                    for pl in range(2):
                        cid = ic * NB + T_BF + 2 * u + pl
                        emit_square(sq8[:, pl, :], xs, cid, rb,
                                    f"{rb}_{uid}_{pl}")
                    bas8 = b8pool.tile([128, 2, RBW], F8, tag="bas8",
                                       name=f"bas8_{rb}_{uid}")
                    nc.scalar.activation(bas8, sq8, AF.Exp, scale=-1.0)
                    mms_dr(bas8, wt8)

                # interleave bf (ACT-heavy) and sc (pool/DVE-heavy) units
                for ic in range(ICHUNK):
                    seq = [("b", 0), ("s", 0), ("s", 1), ("b", 1), ("s", 2),
                           ("s", 3), ("b", 2), ("s", 4), ("b", 3), ("s", 5)]
                    for kind, idx in seq:
                        if kind == "b":
                            bf_unit(ic, idx)
                        else:
                            sc_unit(ic, idx)
                return ps

            def base_and_out(rb, ps):
                ns = rb * RBW
                for mt in range(MT):
                    for bc in range(ICHUNK):
                        last = bc == ICHUNK - 1
                        lhsT = cosx_sb[bc // 2][:, bc % 2, ns + mt * 128 : ns + (mt + 1) * 128]
                        for ob in range(2):
                            nc.tensor.matmul(
                                ps[mt][ob], lhsT,
                                sbt_sb[bc][:, ob * 512 : (ob + 1) * 512],
                                start=False, stop=last,
                            )
                    y_sb = ypool.tile([128, OUT], F32, tag="y",
                                      name=f"y{rb}_{mt}")
                    for ob in range(2):
                        nc.vector.scalar_tensor_tensor(
                            y_sb[:, ob * 512 : (ob + 1) * 512],
                            ps[mt][ob], 1.0 / S,
                            bias_bc[:, ob * 512 : (ob + 1) * 512],
                            ALU.mult, ALU.add,
                        )
                    r0 = ns + mt * 128
                    (nc.sync, nc.gpsimd)[mt % 2].dma_start(
                        out=y[r0 : r0 + 128, :], in_=y_sb
                    )

            # ---- rb0 spline stream ----
            ps0 = spline_units(0)

            # ---- cos path on DVE, [128, 2048] mega-tiles ----
            for g in range(ICHUNK // 2):
                xs = xt_sb[g]
                t1 = tmp.tile([128, 2, NSH], F32, tag="t1", name=f"t1_{g}")
                nc.vector.tensor_scalar_mul(t1, xs, 1.0 / TWO_PI)
                t2 = tmp.tile([128, 2, NSH], F32, tag="t2", name=f"t2_{g}")
                nc.vector.tensor_scalar_add(t2, t1, MAGIC)
                nc.vector.tensor_scalar_sub(t1, t2, MAGIC)
                nc.vector.tensor_scalar_mul(t2, t1, -TWO_PI)
                r = tmp.tile([128, 2, NSH], F32, tag="r", name=f"r_{g}")
                nc.vector.tensor_add(r, xs, t2)
                u = tmp.tile([128, 2, NSH], F32, tag="u", name=f"u_{g}")
                nc.vector.tensor_mul(u, r, r)
                nc.vector.tensor_scalar_mul(t1, u, CC[5])
                nc.vector.scalar_tensor_tensor(t2, t1, CC[4], u, ALU.add, ALU.mult)
                nc.vector.scalar_tensor_tensor(t1, t2, CC[3], u, ALU.add, ALU.mult)
                nc.vector.scalar_tensor_tensor(t2, t1, CC[2], u, ALU.add, ALU.mult)
                nc.vector.scalar_tensor_tensor(t1, t2, CC[1], u, ALU.add, ALU.mult)
                nc.vector.tensor_scalar_add(cosx_sb[g], t1, CC[0])

            # ---- rb0 base + out, rb1 ----
            base_and_out(0, ps0)
            ps1 = spline_units(1)
            base_and_out(1, ps1)

    nc.compile()
    return nc


def _fp8_neighbors(v):
    """Down/up e4m3 grid neighbors of v (f32)."""
    r = v.astype(F8NP).astype(np.float32)
    eps = (np.maximum(np.abs(r), 2.0 ** -9) * 2.0 ** -4).astype(np.float32)
    lo = np.where(r > v, (v - eps).astype(F8NP).astype(np.float32), r)
    hi = np.where(r <= v, (v + eps).astype(F8NP).astype(np.float32), r)
    return np.minimum(lo, r), np.maximum(hi, r)


def _global_prep(inputs):
    """Rank slots, AdaRound fp8 W, pack device arrays (shared by all cores)."""
    x = np.asarray(inputs["x"], np.float32)
    bwf = np.asarray(inputs["basis_w"], np.float32)
    bbf = np.asarray(inputs["basis_b"], np.float32)
    W = np.asarray(inputs["W"], np.float32)
    bias = np.asarray(inputs["bias"], np.float32)
    sbm = np.asarray(inputs["scale_base"], np.float32)

    ns = 1024
    xs = x[:: max(1, x.shape[0] // ns)][:ns]
    t = xs[:, None, :] * bwf[None] + bbf[None]
    b = np.exp(-16.0 * t * t).astype(np.float32)
    b8 = b.astype(F8NP).astype(np.float32)

    Wt = np.ascontiguousarray(W.transpose(1, 2, 0))      # [NB, IN, OUT]
    db2 = ((b8 - b) ** 2).mean(0)                        # [NB, IN]
    sumW2 = (Wt.astype(np.float64) ** 2).sum(2).astype(np.float32)
    mass = db2 * sumW2
    order = np.argsort(-mass, axis=0)                    # [NB, IN]

    Wp = np.take_along_axis(Wt, order[:, :, None], axis=0)
    bwp = np.take_along_axis(bwf, order, axis=0)
    bbp = np.take_along_axis(bbf, order, axis=0)
    b8p = np.take_along_axis(b8, order[None], axis=1)    # [ns, NB, IN]

    # AdaRound for fp8 slots
    WS = (Wp.astype(np.float64) * S).astype(np.float32)
    mhat = b8p.copy()
    mhat[:, :T_BF, :] = 0.0
    m = np.ascontiguousarray(mhat.transpose(2, 1, 0))    # [IN, NB, ns]
    C = np.einsum("ijn,ikn->ijk", m, m, optimize=True) / ns
    Ci = np.ascontiguousarray(C.transpose(1, 2, 0))      # [NB, NB, IN]
    lo, hi = _fp8_neighbors(WS)
    rne = WS.astype(F8NP).astype(np.float32)
    dlo = lo - WS
    dhi = hi - WS
    d = np.where(np.abs(rne - hi) < np.abs(rne - lo), dhi, dlo)
    d[:T_BF] = 0.0
    for _ in range(3):
        for j in range(T_BF, NB):
            s = np.einsum("li,lik->ik", Ci[j], d, optimize=True) \
                - Ci[j, j][:, None] * d[j]
            Cjj = Ci[j, j][:, None]
            cost_lo = Cjj * dlo[j] ** 2 + 2.0 * dlo[j] * s
            cost_hi = Cjj * dhi[j] ** 2 + 2.0 * dhi[j] * s
            d[j] = np.where(cost_hi < cost_lo, dhi[j], dlo[j])
    W8 = WS + d                                          # fp8 grid values

    # pack unit streams (ic-major)
    w_bf = np.empty((ICHUNK * T_BF * 128, OUT), dtype=ml_dtypes.bfloat16)
    for ic in range(ICHUNK):
        blk = slice(ic * 128, (ic + 1) * 128)
        for t_ in range(T_BF):
            uid = ic * T_BF + t_
            w_bf[uid * 128 : (uid + 1) * 128] = (
                Wp[t_, blk, :] * np.float32(S)
            ).astype(ml_dtypes.bfloat16)
    w_f8 = np.empty((ICHUNK * NU * 256, OUT), dtype=F8NP)
    for ic in range(ICHUNK):
        blk = slice(ic * 128, (ic + 1) * 128)
        for u in range(NU):
            uid = ic * NU + u
            for pl in range(2):
                rows = slice(uid * 256 + pl * 128, uid * 256 + (pl + 1) * 128)
                w_f8[rows] = W8[T_BF + 2 * u + pl, blk, :].astype(F8NP)

    # per-chunk scale/bias tables: cid = ic*NB + slot
    bw4 = np.empty((128, NCHK), np.float32)
    bb4 = np.empty((128, NCHK), np.float32)
    for ic in range(ICHUNK):
        blk = slice(ic * 128, (ic + 1) * 128)
        for slot in range(NB):
            cid = ic * NB + slot
            bw4[:, cid] = 4.0 * bwp[slot, blk]
            bb4[:, cid] = 4.0 * bbp[slot, blk]

    sb_t = np.ascontiguousarray(sbm.T * np.float32(S)).astype(ml_dtypes.bfloat16)
    bias_f = np.ascontiguousarray(bias.reshape(1, OUT))
    return {
        "w_bf": w_bf,
        "w_f8": w_f8,
        "sb_t": sb_t,
        "bw4": np.ascontiguousarray(bw4),
        "bb4": np.ascontiguousarray(bb4),
        "bias_f": bias_f,
    }


def _prep(inputs):
    key = None
    xarr = np.asarray(inputs["x"])
    key = (xarr.shape, xarr.dtype.str, xarr[:2, :4].tobytes())
    if _cache.get("prep_key") != key:
        _cache["prep"] = _global_prep(inputs)
        _cache["prep_key"] = key
    shared = _cache["prep"]

    x = np.asarray(inputs["x"], dtype=np.float32)
    in_maps = []
    for c in range(N_CORES):
        shard = x[c * NSH : (c + 1) * NSH, :]
        x_t = np.ascontiguousarray(shard.T)
        im = {"x_t": x_t}
        im.update(shared)
        in_maps.append(im)
    return in_maps


def run(inputs, trace=False, **kw):
    if "nc" not in _cache:
        _cache["nc"] = _build()
    nc = _cache["nc"]
    in_maps = _prep(inputs)
    res = run_bass_kernel_spmd(
        nc, in_maps, core_ids=list(range(N_CORES)), trace=trace, **kw
    )
    out = np.concatenate([res.results[c]["y"] for c in range(N_CORES)], axis=0)
    return out, res


def kernel(**inputs) -> np.ndarray:
    out, _ = run(inputs, trace=False)
    return out


# revision 19
# speedup vs baseline: 1.0168x; 1.0129x over previous
"""KAN layer kernel for 8x Trainium2 NeuronCores — fp8-DoubleRow hybrid.

y[n,k] = sum_{j,i} exp(-16*(x[n,i]*bw[j,i]+bb[j,i])^2) * W[k,j,i]
         + bias[k] + cos(x) @ scale_base.T

Sharding: data-parallel over N (8192 rows -> 1024 rows/core), params
replicated.

Precision plan (per input feature i, the 16 j-basis pairs are ranked by
fp8-quantization error mass, estimated on a row subsample):
  - top T=4 slots  -> bf16 chunks (exact-ish), standard matmuls
  - remaining 12   -> fp8 e4m3, packed 2 slots/super-chunk, DoubleRow
    matmuls (2x contraction per PE pass)
  - W for fp8 slots is quantized with batch-adaptive coordinated rounding
    (AdaRound-style greedy minimizing E_n[(sum_j bhat_j dW_j)^2] per (i,k))
  - base path cos(x) @ scale_base.T stays bf16
All matmuls accumulate f32 into shared PSUM banks; W (and scale_base) are
pre-scaled by S=1024 so fp8 W values sit in e4m3's normal range; the 1/S
dequant is folded into the PSUM->SBUF copy.

Engine balance: PE is the bottleneck (~306us). Exp passes must run on ACT;
Square passes are routed across ACT / Pool(+DVE) / Pool(+Pool) so no
helper engine exceeds the PE time. cos runs on DVE in [128,2048] passes.
"""

import sys

for _p in ("/opt/trn_rl_repo",):
    if _p not in sys.path:
        sys.path.insert(0, _p)

import math

import ml_dtypes
import numpy as np

import concourse.bass as bass
import concourse.mybir as mybir
import concourse.tile as tile
from concourse import bacc
from concourse.bass_utils import run_bass_kernel_spmd

F32 = mybir.dt.float32
BF16 = mybir.dt.bfloat16
F8 = mybir.dt.float8e4
AF = mybir.ActivationFunctionType
ALU = mybir.AluOpType
PM = mybir.MatmulPerfMode
F8NP = ml_dtypes.float8_e4m3

N_CORES = 8
N, IN, OUT, NB = 8192, 1024, 1024, 16
NSH = N // N_CORES            # rows per core = 1024
ICHUNK = IN // 128            # 8 i-chunks
RB = 2                        # row blocks per core
RBW = NSH // RB               # 512 rows per block
MT = RBW // 128               # 4 m-tiles per block
S = 1024.0                    # fp8 W scale

T_BF = 4                      # bf16 slots per i
NU = (NB - T_BF) // 2         # fp8 super-chunk units per ic = 6
NCHK = NB * ICHUNK            # 128 chunk-slots total

TWO_PI = 2.0 * math.pi
MAGIC = 12582912.0            # 1.5 * 2**23
CC = [
    0.9999992107823226,
    -0.49999421338471783,
    0.04165977780655192,
    -0.0013858789919604375,
    2.420294136739255e-05,
    -2.1972963819539338e-07,
]

# square-pass routing for fp8 chunks: A = ACT Square; PD = Pool affine +
# DVE mult; PP = Pool affine + Pool mult
ROUTE_PATTERN = ("PD", "PP", "PD", "A")

_cache = {}


def _route(sc_chunk_idx):
    return ROUTE_PATTERN[sc_chunk_idx % len(ROUTE_PATTERN)]


def _build():
    nc = bacc.Bacc("TRN2", target_bir_lowering=False)

    x_t = nc.dram_tensor("x_t", [IN, NSH], F32, kind="ExternalInput")
    w_bf = nc.dram_tensor("w_bf", [ICHUNK * T_BF * 128, OUT], BF16,
                          kind="ExternalInput")
    w_f8 = nc.dram_tensor("w_f8", [ICHUNK * NU * 256, OUT], F8,
                          kind="ExternalInput")
    sb_t = nc.dram_tensor("sb_t", [IN, OUT], BF16, kind="ExternalInput")
    bw4 = nc.dram_tensor("bw4", [128, NCHK], F32, kind="ExternalInput")
    bb4 = nc.dram_tensor("bb4", [128, NCHK], F32, kind="ExternalInput")
    bias_f = nc.dram_tensor("bias_f", [1, OUT], F32, kind="ExternalInput")
    y = nc.dram_tensor("y", [NSH, OUT], F32, kind="ExternalOutput")

    with tile.TileContext(nc) as tc:
        with (
            tc.tile_pool(name="singles", bufs=1) as singles,
            tc.tile_pool(name="wpool", bufs=5) as wpool,
            tc.tile_pool(name="w8pool", bufs=5) as w8pool,
            tc.tile_pool(name="sqpool", bufs=3) as sqpool,
            tc.tile_pool(name="sq8pool", bufs=3) as sq8pool,
            tc.tile_pool(name="tpool", bufs=3) as tpool,
            tc.tile_pool(name="bpool", bufs=3) as bpool,
            tc.tile_pool(name="b8pool", bufs=3) as b8pool,
            tc.tile_pool(name="ypool", bufs=3) as ypool,
            tc.tile_pool(name="tmp", bufs=1) as tmp,
            tc.tile_pool(name="psum", bufs=1, space="PSUM") as psum,
        ):
            # tiny per-chunk scale/bias tables first on the sync ring
            bw4_sb = singles.tile([128, NCHK], F32)
            nc.sync.dma_start(out=bw4_sb, in_=bw4[:])
            bb4_sb = singles.tile([128, NCHK], F32)
            nc.sync.dma_start(out=bb4_sb, in_=bb4[:])

            # x^T resident as one tile per PAIR of i-chunks so consumers only
            # wait on their own pair's DMAs (a single 4MB tile serializes the
            # first Square behind the whole x load); pairs keep the cos path's
            # [128, 2048] DVE passes possible
            xt_dram = x_t[:].rearrange("(c p) n -> p c n", p=128)
            xt_sb = []
            for g in range(ICHUNK // 2):
                xt_sb.append(singles.tile([128, 2, NSH], F32, name=f"xt{g}"))

            def xs_of(ic):
                return xt_sb[ic // 2][:, ic % 2, :]

            # only the immediately-needed transfers go on compute queues
            # (one DMA each, no flow-control wait): xt0b on scalar, xt1 on
            # gpsimd. Everything else rides sync, the side DMAs emitted
            # lazily between units to interleave with the W stream.
            nc.sync.dma_start(out=xt_sb[0][:, 0, :RBW], in_=xt_dram[:, 0, :RBW])
            nc.scalar.dma_start(out=xt_sb[0][:, 0, RBW:], in_=xt_dram[:, 0, RBW:])
            nc.gpsimd.dma_start(out=xs_of(1), in_=xt_dram[:, 1, :])

            sbt_dram = sb_t[:].rearrange("(c p) n -> p c n", p=128)
            sbt_sb = []
            for c in range(ICHUNK):
                sbt_sb.append(singles.tile([128, OUT], BF16, name=f"sbt{c}"))
            bias_bc = singles.tile([128, OUT], F32)
            cosx_sb = []
            for g in range(ICHUNK // 2):
                cosx_sb.append(singles.tile([128, 2, NSH], BF16, name=f"cosx{g}"))

            side_dmas = (
                [(xs_of(ic), xt_dram[:, ic, :]) for ic in range(2, ICHUNK)]
                + [(sbt_sb[c], sbt_dram[:, c, :]) for c in range(ICHUNK)]
                + [(bias_bc, bias_f[:].to_broadcast([128, OUT]))]
            )
            side_state = [0]

            def emit_side_dma():
                if side_state[0] < len(side_dmas):
                    dst, src = side_dmas[side_state[0]]
                    side_state[0] += 1
                    nc.sync.dma_start(out=dst, in_=src)

            sc_sq_counter = [0]

            def emit_square(dst, xs, cid, rb, name):
                """dst[128, RBW] f32 = (bw4*x + bb4)^2 via routed engines."""
                r = _route(sc_sq_counter[0])
                sc_sq_counter[0] += 1
                if r == "A":
                    nc.scalar.activation(
                        dst, xs, AF.Square,
                        bias=bb4_sb[:, cid : cid + 1],
                        scale=bw4_sb[:, cid : cid + 1],
                    )
                else:
                    t = tpool.tile([128, RBW], F32, tag="t", name=f"t{name}")
                    nc.gpsimd.tensor_scalar(
                        t, xs, bw4_sb[:, cid : cid + 1],
                        bb4_sb[:, cid : cid + 1], ALU.mult, ALU.add,
                    )
                    eng = nc.vector if r == "PD" else nc.gpsimd
                    eng.tensor_tensor(dst, t, t, ALU.mult)

            def spline_units(rb):
                ns = rb * RBW
                ps = [
                    [
                        psum.tile(
                            [128, 512], F32,
                            tag=f"ps_{mt}_{ob}", name=f"ps_{rb}_{mt}_{ob}",
                        )
                        for ob in range(2)
                    ]
                    for mt in range(MT)
                ]
                first = [True]

                def mms_bf(bas, wt):
                    for mt in range(MT):
                        lhsT = bas[:, mt * 128 : (mt + 1) * 128]
                        for ob in range(2):
                            nc.tensor.matmul(
                                ps[mt][ob], lhsT,
                                wt[:, ob * 512 : (ob + 1) * 512],
                                start=first[0], stop=False,
                            )
                    first[0] = False

                def mms_dr(bas8, wt8):
                    for mt in range(MT):
                        lhsT = bas8[:, :, mt * 128 : (mt + 1) * 128]
                        for ob in range(2):
                            nc.tensor.matmul(
                                ps[mt][ob], lhsT,
                                wt8[:, :, ob * 512 : (ob + 1) * 512],
                                start=first[0], stop=False,
                                perf_mode=PM.DoubleRow,
                            )
                    first[0] = False

                def bf_unit(ic, t_):
                    uid = ic * T_BF + t_
                    cid = ic * NB + t_
                    wt = wpool.tile([128, OUT], BF16, tag="wt",
                                    name=f"wt{rb}_{uid}")
                    nc.sync.dma_start(
                        out=wt, in_=w_bf[uid * 128 : (uid + 1) * 128, :]
                    )
                    sq = sqpool.tile([128, RBW], F32, tag="sq",
                                     name=f"sq{rb}_{uid}")
                    nc.scalar.activation(
                        sq, xs_of(ic)[:, ns : ns + RBW], AF.Square,
                        bias=bb4_sb[:, cid : cid + 1],
                        scale=bw4_sb[:, cid : cid + 1],
                    )
                    bas = bpool.tile([128, RBW], BF16, tag="bas",
                                     name=f"bas{rb}_{uid}")
                    nc.scalar.activation(bas, sq, AF.Exp, scale=-1.0)
                    mms_bf(bas, wt)

                def sc_unit(ic, u):
                    uid = ic * NU + u
                    wt8 = w8pool.tile([128, 2, OUT], F8, tag="wt8",
                                      name=f"wt8_{rb}_{uid}")
                    nc.sync.dma_start(
                        out=wt8,
                        in_=w_f8[uid * 256 : (uid + 1) * 256, :].rearrange(
                            "(two p) o -> p two o", two=2
                        ),
                    )
                    sq8 = sq8pool.tile([128, 2, RBW], F32, tag="sq8",
                                       name=f"sq8_{rb}_{uid}")
                    xs = xs_of(ic)[:, ns : ns + RBW]
                    for pl in range(2):
                        cid = ic * NB + T_BF + 2 * u + pl
                        emit_square(sq8[:, pl, :], xs, cid, rb,
                                    f"{rb}_{uid}_{pl}")
                    bas8 = b8pool.tile([128, 2, RBW], F8, tag="bas8",
                                       name=f"bas8_{rb}_{uid}")
                    nc.scalar.activation(bas8, sq8, AF.Exp, scale=-1.0)
                    mms_dr(bas8, wt8)

                # interleave bf (ACT-heavy) and sc (pool/DVE-heavy) units
                for ic in range(ICHUNK):
                    seq = [("b", 0), ("s", 0), ("s", 1), ("b", 1), ("s", 2),
                           ("s", 3), ("b", 2), ("s", 4), ("b", 3), ("s", 5)]
                    for kind, idx in seq:
                        if kind == "b":
                            bf_unit(ic, idx)
                        else:
                            sc_unit(ic, idx)
                        if rb == 0:
                            emit_side_dma()
                return ps

            def base_and_out(rb, ps):
                ns = rb * RBW
                for mt in range(MT):
                    for bc in range(ICHUNK):
                        last = bc == ICHUNK - 1
                        lhsT = cosx_sb[bc // 2][:, bc % 2, ns + mt * 128 : ns + (mt + 1) * 128]
                        for ob in range(2):
                            nc.tensor.matmul(
                                ps[mt][ob], lhsT,
                                sbt_sb[bc][:, ob * 512 : (ob + 1) * 512],
                                start=False, stop=last,
                            )
                    y_sb = ypool.tile([128, OUT], F32, tag="y",
                                      name=f"y{rb}_{mt}")
                    for ob in range(2):
                        nc.vector.scalar_tensor_tensor(
                            y_sb[:, ob * 512 : (ob + 1) * 512],
                            ps[mt][ob], 1.0 / S,
                            bias_bc[:, ob * 512 : (ob + 1) * 512],
                            ALU.mult, ALU.add,
                        )
                    r0 = ns + mt * 128
                    nc.sync.dma_start(out=y[r0 : r0 + 128, :], in_=y_sb)

            # ---- rb0 spline stream ----
            ps0 = spline_units(0)

            # ---- cos path on DVE, [128, 2048] mega-tiles ----
            for g in range(ICHUNK // 2):
                xs = xt_sb[g]
                t1 = tmp.tile([128, 2, NSH], F32, tag="t1", name=f"t1_{g}")
                nc.vector.tensor_scalar_mul(t1, xs, 1.0 / TWO_PI)
                t2 = tmp.tile([128, 2, NSH], F32, tag="t2", name=f"t2_{g}")
                nc.vector.tensor_scalar_add(t2, t1, MAGIC)
                nc.vector.tensor_scalar_sub(t1, t2, MAGIC)
                nc.vector.tensor_scalar_mul(t2, t1, -TWO_PI)
                r = tmp.tile([128, 2, NSH], F32, tag="r", name=f"r_{g}")
                nc.vector.tensor_add(r, xs, t2)
                u = tmp.tile([128, 2, NSH], F32, tag="u", name=f"u_{g}")
                nc.vector.tensor_mul(u, r, r)
                nc.vector.tensor_scalar_mul(t1, u, CC[5])
                nc.vector.scalar_tensor_tensor(t2, t1, CC[4], u, ALU.add, ALU.mult)
                nc.vector.scalar_tensor_tensor(t1, t2, CC[3], u, ALU.add, ALU.mult)
                nc.vector.scalar_tensor_tensor(t2, t1, CC[2], u, ALU.add, ALU.mult)
                nc.vector.scalar_tensor_tensor(t1, t2, CC[1], u, ALU.add, ALU.mult)
                nc.vector.tensor_scalar_add(cosx_sb[g], t1, CC[0])

            # ---- rb0 base + out, rb1 ----
            base_and_out(0, ps0)
            ps1 = spline_units(1)
            base_and_out(1, ps1)

    nc.compile()
    return nc


def _fp8_neighbors(v):
    """Down/up e4m3 grid neighbors of v (f32)."""
    r = v.astype(F8NP).astype(np.float32)
    eps = (np.maximum(np.abs(r), 2.0 ** -9) * 2.0 ** -4).astype(np.float32)
    lo = np.where(r > v, (v - eps).astype(F8NP).astype(np.float32), r)
    hi = np.where(r <= v, (v + eps).astype(F8NP).astype(np.float32), r)
    return np.minimum(lo, r), np.maximum(hi, r)


def _global_prep(inputs):
    """Rank slots, AdaRound fp8 W, pack device arrays (shared by all cores)."""
    x = np.asarray(inputs["x"], np.float32)
    bwf = np.asarray(inputs["basis_w"], np.float32)
    bbf = np.asarray(inputs["basis_b"], np.float32)
    W = np.asarray(inputs["W"], np.float32)
    bias = np.asarray(inputs["bias"], np.float32)
    sbm = np.asarray(inputs["scale_base"], np.float32)

    ns = 1024
    xs = x[:: max(1, x.shape[0] // ns)][:ns]
    t = xs[:, None, :] * bwf[None] + bbf[None]
    b = np.exp(-16.0 * t * t).astype(np.float32)
    b8 = b.astype(F8NP).astype(np.float32)

    Wt = np.ascontiguousarray(W.transpose(1, 2, 0))      # [NB, IN, OUT]
    db2 = ((b8 - b) ** 2).mean(0)                        # [NB, IN]
    sumW2 = (Wt.astype(np.float64) ** 2).sum(2).astype(np.float32)
    mass = db2 * sumW2
    order = np.argsort(-mass, axis=0)                    # [NB, IN]

    Wp = np.take_along_axis(Wt, order[:, :, None], axis=0)
    bwp = np.take_along_axis(bwf, order, axis=0)
    bbp = np.take_along_axis(bbf, order, axis=0)
    b8p = np.take_along_axis(b8, order[None], axis=1)    # [ns, NB, IN]

    # AdaRound for fp8 slots
    WS = (Wp.astype(np.float64) * S).astype(np.float32)
    mhat = b8p.copy()
    mhat[:, :T_BF, :] = 0.0
    m = np.ascontiguousarray(mhat.transpose(2, 1, 0))    # [IN, NB, ns]
    C = np.einsum("ijn,ikn->ijk", m, m, optimize=True) / ns
    Ci = np.ascontiguousarray(C.transpose(1, 2, 0))      # [NB, NB, IN]
    lo, hi = _fp8_neighbors(WS)
    rne = WS.astype(F8NP).astype(np.float32)
    dlo = lo - WS
    dhi = hi - WS
    d = np.where(np.abs(rne - hi) < np.abs(rne - lo), dhi, dlo)
    d[:T_BF] = 0.0
    for _ in range(3):
        for j in range(T_BF, NB):
            s = np.einsum("li,lik->ik", Ci[j], d, optimize=True) \
                - Ci[j, j][:, None] * d[j]
            Cjj = Ci[j, j][:, None]
            cost_lo = Cjj * dlo[j] ** 2 + 2.0 * dlo[j] * s
            cost_hi = Cjj * dhi[j] ** 2 + 2.0 * dhi[j] * s
            d[j] = np.where(cost_hi < cost_lo, dhi[j], dlo[j])
    W8 = WS + d                                          # fp8 grid values

    # pack unit streams (ic-major)
    w_bf = np.empty((ICHUNK * T_BF * 128, OUT), dtype=ml_dtypes.bfloat16)
    for ic in range(ICHUNK):
        blk = slice(ic * 128, (ic + 1) * 128)
        for t_ in range(T_BF):
            uid = ic * T_BF + t_
            w_bf[uid * 128 : (uid + 1) * 128] = (
                Wp[t_, blk, :] * np.float32(S)
            ).astype(ml_dtypes.bfloat16)
    w_f8 = np.empty((ICHUNK * NU * 256, OUT), dtype=F8NP)
    for ic in range(ICHUNK):
        blk = slice(ic * 128, (ic + 1) * 128)
        for u in range(NU):
            uid = ic * NU + u
            for pl in range(2):
                rows = slice(uid * 256 + pl * 128, uid * 256 + (pl + 1) * 128)
                w_f8[rows] = W8[T_BF + 2 * u + pl, blk, :].astype(F8NP)

    # per-chunk scale/bias tables: cid = ic*NB + slot
    bw4 = np.empty((128, NCHK), np.float32)
    bb4 = np.empty((128, NCHK), np.float32)
    for ic in range(ICHUNK):
        blk = slice(ic * 128, (ic + 1) * 128)
        for slot in range(NB):
            cid = ic * NB + slot
            bw4[:, cid] = 4.0 * bwp[slot, blk]
            bb4[:, cid] = 4.0 * bbp[slot, blk]

    sb_t = np.ascontiguousarray(sbm.T * np.float32(S)).astype(ml_dtypes.bfloat16)
    bias_f = np.ascontiguousarray(bias.reshape(1, OUT))
    return {
        "w_bf": w_bf,
        "w_f8": w_f8,
        "sb_t": sb_t,
        "bw4": np.ascontiguousarray(bw4),
        "bb4": np.ascontiguousarray(bb4),
        "bias_f": bias_f,
    }


def _prep(inputs):
    key = None
    xarr = np.asarray(inputs["x"])
    key = (xarr.shape, xarr.dtype.str, xarr[:2, :4].tobytes())
    if _cache.get("prep_key") != key:
        _cache["prep"] = _global_prep(inputs)
        _cache["prep_key"] = key
    shared = _cache["prep"]

    x = np.asarray(inputs["x"], dtype=np.float32)
    in_maps = []
    for c in range(N_CORES):
        shard = x[c * NSH : (c + 1) * NSH, :]
        x_t = np.ascontiguousarray(shard.T)
        im = {"x_t": x_t}
        im.update(shared)
        in_maps.append(im)
    return in_maps


def run(inputs, trace=False, **kw):
    if "nc" not in _cache:
        _cache["nc"] = _build()
    nc = _cache["nc"]
    in_maps = _prep(inputs)
    res = run_bass_kernel_spmd(
        nc, in_maps, core_ids=list(range(N_CORES)), trace=trace, **kw
    )
    out = np.concatenate([res.results[c]["y"] for c in range(N_CORES)], axis=0)
    return out, res


def kernel(**inputs) -> np.ndarray:
    out, _ = run(inputs, trace=False)
    return out
